# revision 8
# baseline (speedup 1.0000x reference)
"""Trainium2 Bass kernel for NanochatAttention (sliding-window GQA attention).

Sharding: 8 cores = (batch b in {0,1}) x (kv-group g in {0..3}).
Each core handles one batch's full sequence for one KV head and its 4 Q heads:
projections, RoPE + QK RMS-norm, value-embedding gate, 512-window causal
attention, and the row-parallel out-projection slice -> partial [T, E] output.
Host sums the 4 partials per batch at unshard time.

Phases (keeps the PE clock warm and ACT tables stable):
  A: projections (PSUM) -> RoPE/RMS stats -> q/k transposes to [d, t].
     Input DMAs are split per contraction slab so the first projection chain
     rides the DMA wave instead of waiting for the full 11 MB.
  B+C merged: attention per (t-tile, head) -- scores + additive-mask matmuls
     -> exp (scale = per-row qscale, row sums via accum) -> P transposed AND
     divided by rowsum in one matmul against diag(1/s) -> PV -> yT [d, t];
     then immediately the out-projection for the finished t-tile (dense PE
     filler work that hides the exp/copy latency chain) -> DMA out.
"""

import numpy as np
import ml_dtypes

import concourse.bass as bass
import concourse.bacc as bacc
import concourse.tile as tile
from concourse import mybir
from concourse import bass_utils

BF = mybir.dt.bfloat16
F32 = mybir.dt.float32
AF = mybir.ActivationFunctionType
ALU = mybir.AluOpType

B = 2
T = 2048
E = 2048
D = 128          # head dim
HQ = 4           # q heads per core (one kv group)
NKV = 4
NT = T // 128    # 16 t-tiles
NE = E // 128    # 16 e-tiles
W = 512          # sliding window
NJB = W // 128   # history blocks
EPS = float(np.finfo(np.float32).eps)
SQRT_D = float(np.sqrt(128.0))


def _bcast_mid(ap, n):
    """Insert a step-0 dim after the partition dim: [p, w] -> [p, n, w]."""
    return bass.AP(tensor=ap.tensor, offset=ap.offset,
                   ap=[ap.ap[0], [0, n], *ap.ap[1:]])


def _half_swap(ap2d, nmid):
    """[p, nmid*128] -> [p, nmid, 2, 64] view with the 64-halves swapped."""
    return bass.AP(tensor=ap2d.tensor, offset=ap2d.offset + 64,
                   ap=[ap2d.ap[0], [128, nmid], [-64, 2], [1, 64]])


def _body(tc, io):
    nc = tc.nc
    xT, wq, wkvg, wo, ve2, cosd, sind, masks, ident, out = (
        io["xT"], io["wq"], io["wkvg"], io["wo"], io["ve2"], io["cos"],
        io["sin"], io["masks"], io["ident"], io["out"])

    with (
        tc.tile_pool(name="const", bufs=1) as cpool,
        tc.tile_pool(name="state", bufs=1) as state,
    ):
        # ---- small constants first (unblock tile-0 work) ----------------
        ident_sb = cpool.tile([128, 128], BF, tag="ident")
        nc.sync.dma_start(ident_sb, ident)
        masks_sb = cpool.tile([128, 2, 128], BF, tag="masks")
        nc.sync.dma_start(masks_sb, masks.rearrange("m p j -> p m j"))
        # cos_ext = [cos, cos], sin_ext = [sin, -sin]  (for 3-op RoPE)
        cos_sb = cpool.tile([128, NT, 128], F32, tag="cos")
        nc.sync.dma_start(cos_sb, cosd.rearrange("(t p) h -> p t h", p=128))
        sin_sb = cpool.tile([128, NT, 128], F32, tag="sin")
        nc.sync.dma_start(sin_sb, sind.rearrange("(t p) h -> p t h", p=128))

        # ---- per-slab weight + activation loads (pipelined) --------------
        wq_sb = cpool.tile([128, NE, HQ * D], BF, tag="wq")
        wkvg_sb = cpool.tile([128, NE, 257], BF, tag="wkvg")
        with tc.tile_pool(name="xp", bufs=1) as xp:
            xT_sb = xp.tile([128, NE, T], BF, tag="xT")
            for e in range(NE):
                es = slice(e * 128, (e + 1) * 128)
                nc.sync.dma_start(wq_sb[:, e, :], wq[es, :])
                nc.sync.dma_start(wkvg_sb[:, e, :], wkvg[es, :])
                nc.sync.dma_start(xT_sb[:, e, :], xT[es, :])

            ve_sb = cpool.tile([128, NT, D], BF, tag="ve")
            nc.sync.dma_start(ve_sb, ve2.rearrange("(t p) d -> p t d", p=128))
            wo_sb = cpool.tile([128, HQ, E], BF, tag="wo")
            nc.sync.dma_start(wo_sb, wo.rearrange("(h p) e -> p h e", p=128))

            # ---- whole-sequence state ------------------------------------
            qkT_all = state.tile([128, NT, HQ + 1, 128], BF, tag="qkT")
            v_all = state.tile([128, NT, D], BF, tag="v")
            yT_all = state.tile([128, NT, HQ, 128], BF, tag="yT")
            sc_all = state.tile([128, NT, HQ + 1], F32, tag="sc")
            vstg = state.tile([128, NT, 129], BF, tag="vstg")

            # ============ PHASE A: proj + rope + rms + transposes =========
            with (
                tc.tile_pool(name="workA", bufs=2) as work,
                tc.tile_pool(name="psA_q", bufs=2, space="PSUM") as psA_q,
                tc.tile_pool(name="psA_kvg", bufs=2, space="PSUM") as psA_kvg,
                tc.tile_pool(name="psA_tr", bufs=2, space="PSUM") as psA_tr,
            ):
                for tt in range(NT):
                    ts = slice(tt * 128, (tt + 1) * 128)
                    psq = psA_q.tile([128, HQ * D], F32, tag="psq")
                    pskvg = psA_kvg.tile([128, 257], F32, tag="pskvg")
                    for e in range(NE):
                        nc.tensor.matmul(psq, xT_sb[:, e, ts], wq_sb[:, e, :],
                                         start=(e == 0), stop=(e == NE - 1))
                    for e in range(NE):
                        nc.tensor.matmul(pskvg, xT_sb[:, e, ts],
                                         wkvg_sb[:, e, :],
                                         start=(e == 0), stop=(e == NE - 1))

                    # RoPE: ro = q*cosE + halfswap(q)*sinE   (3 ops each)
                    cosq = _bcast_mid(cos_sb[:, tt, :], HQ)
                    sinq = _bcast_mid(sin_sb[:, tt, :], HQ)
                    qro = work.tile([128, HQ, D], F32, tag="qro")
                    rb = work.tile([128, HQ, D], F32, tag="rb")
                    nc.vector.tensor_mul(
                        qro, psq.rearrange("p (h d) -> p h d", h=HQ), cosq)
                    nc.vector.tensor_mul(
                        rb.rearrange("p h (s x) -> p h s x", s=2),
                        _half_swap(psq[:], HQ), sinq.rearrange(
                            "p h (s x) -> p h s x", s=2))
                    nc.vector.tensor_add(qro, qro, rb)

                    kro = work.tile([128, D], F32, tag="kro")
                    kb = work.tile([128, D], F32, tag="kb")
                    nc.vector.tensor_mul(kro, pskvg[:, 0:128],
                                         cos_sb[:, tt, :])
                    nc.vector.tensor_mul(
                        kb.rearrange("p (s x) -> p s x", s=2),
                        _half_swap(pskvg[:, 0:128], 1),
                        sin_sb[:, tt, :].rearrange("p (s x) -> p s x", s=2))
                    nc.vector.tensor_add(kro, kro, kb)

                    # RMS sums of squares (ACT, one table) then rsqrt
                    sq = work.tile([128, (HQ + 1) * D], F32, tag="sq")
                    qkss = work.tile([128, HQ + 1], F32, tag="qkss")
                    for h in range(HQ):
                        nc.scalar.activation(sq[:, h * D:(h + 1) * D],
                                             qro[:, h, :], AF.Square,
                                             accum_out=qkss[:, h:h + 1])
                    nc.scalar.activation(sq[:, HQ * D:], kro, AF.Square,
                                         accum_out=qkss[:, HQ:HQ + 1])
                    tmp5 = work.tile([128, HQ + 1], F32, tag="tmp5")
                    nc.vector.tensor_scalar_add(tmp5, qkss,
                                                float(128.0 * EPS))
                    rec5 = work.tile([128, HQ + 1], F32, tag="rec5")
                    nc.vector.reciprocal(rec5, tmp5)
                    nc.scalar.activation(sc_all[:, tt, :], rec5, AF.Sqrt)

                    # krms = kro * sc_k * sqrt(128)  (bf16)
                    krms = work.tile([128, D], BF, tag="krms")
                    nc.vector.tensor_scalar(krms, kro,
                                            sc_all[:, tt, HQ:HQ + 1],
                                            SQRT_D, op0=ALU.mult,
                                            op1=ALU.mult)
                    # stage v-proj + gate logit (cols 128:257, contiguous)
                    nc.vector.tensor_copy(vstg[:, tt, :], pskvg[:, 128:257])

                    # cast q to bf16 (gpsimd), transpose q/k to [d, t] on PE
                    qbf = work.tile([128, HQ * D], BF, tag="qbf")
                    nc.gpsimd.tensor_copy(qbf,
                                          qro.rearrange("p h d -> p (h d)"))
                    trp = psA_tr.tile([128, HQ + 1, 128], F32, tag="trp")
                    for h in range(HQ):
                        nc.tensor.matmul(trp[:, h, :],
                                         qbf[:, h * D:(h + 1) * D],
                                         ident_sb, start=True, stop=True)
                    nc.tensor.matmul(trp[:, HQ, :], krms, ident_sb,
                                     start=True, stop=True)
                    nc.vector.tensor_copy(
                        qkT_all[:, tt, :, :].rearrange("p a b -> p (a b)"),
                        trp.rearrange("p a b -> p (a b)"))

        # ---- A->B boundary: batched sigmoid + v assembly ----------------
        with tc.tile_pool(name="bnd", bufs=1) as bnd:
            sig_all = bnd.tile([128, NT], F32, tag="sig")
            nc.scalar.activation(sig_all, vstg[:, :, 128], AF.Sigmoid)
            for tt in range(NT):
                nc.vector.scalar_tensor_tensor(
                    v_all[:, tt, :], ve_sb[:, tt, :], sig_all[:, tt:tt + 1],
                    vstg[:, tt, 0:128], op0=ALU.mult, op1=ALU.add)

            # ========= PHASE B+C: attention + out-projection =============
            with (
                tc.tile_pool(name="attn", bufs=4) as attn,
                tc.tile_pool(name="outc", bufs=2) as outc,
                tc.tile_pool(name="psB", bufs=3, space="PSUM") as psB,
                tc.tile_pool(name="psC", bufs=2, space="PSUM") as psC,
            ):
                for tt in range(NT):
                    njb = min(tt, NJB) + 1
                    jb0 = tt - (njb - 1)
                    nhist = njb - 1
                    for h in range(HQ):
                        s_ps = psB.tile([128, NJB + 1, 128], F32, tag="spt")
                        qT_h = qkT_all[:, tt, h, :]
                        if tt >= NJB:
                            nc.tensor.matmul(s_ps[:, 0, :], qT_h,
                                             qkT_all[:, jb0, HQ, :],
                                             start=True, stop=False)
                            nc.tensor.matmul(s_ps[:, 0, :], ident_sb,
                                             masks_sb[:, 0, :],
                                             start=False, stop=True)
                            if nhist > 1:
                                nc.tensor.matmul(
                                    s_ps[:, 1:nhist, :].rearrange(
                                        "p a b -> p (a b)"),
                                    qT_h,
                                    qkT_all[:, jb0 + 1:tt, HQ, :],
                                    start=True, stop=True)
                        elif nhist > 0:
                            nc.tensor.matmul(
                                s_ps[:, 0:nhist, :].rearrange(
                                    "p a b -> p (a b)"),
                                qT_h,
                                qkT_all[:, jb0:tt, HQ, :],
                                start=True, stop=True)
                        nc.tensor.matmul(s_ps[:, nhist, :], qT_h,
                                         qkT_all[:, tt, HQ, :],
                                         start=True, stop=False)
                        nc.tensor.matmul(s_ps[:, nhist, :], ident_sb,
                                         masks_sb[:, 1, :],
                                         start=False, stop=True)

                        pexp = attn.tile([128, NJB + 1, 128], BF, tag="pexp")
                        ssum = attn.tile([128, 1], F32, tag="ssum")
                        nc.scalar.activation(
                            pexp[:, 0:njb, :].rearrange("p a b -> p (a b)"),
                            s_ps[:, 0:njb, :].rearrange("p a b -> p (a b)"),
                            AF.Exp, scale=sc_all[:, tt, h:h + 1],
                            accum_out=ssum)
                        rsum = attn.tile([128, 1], F32, tag="rsum")
                        nc.vector.reciprocal(rsum, ssum)
                        diag = attn.tile([128, 128], BF, tag="diag")
                        nc.vector.tensor_scalar_mul(diag, ident_sb, rsum)

                        # PT[j, i] = P[i, j]/s(i): overwrite S slot per block
                        for jb in range(njb):
                            nc.tensor.matmul(s_ps[:, jb, :], pexp[:, jb, :],
                                             diag, start=True, stop=True)
                        pt_sb = attn.tile([128, NJB + 1, 128], BF, tag="ptsb")
                        nc.vector.tensor_copy(
                            pt_sb[:, 0:njb, :].rearrange("p a b -> p (a b)"),
                            s_ps[:, 0:njb, :].rearrange("p a b -> p (a b)"))
                        # PV: yT[d, i] accumulated over j-blocks in slot bank0
                        for jb in range(njb):
                            nc.tensor.matmul(s_ps[:, 0, :],
                                             v_all[:, jb0 + jb, :],
                                             pt_sb[:, jb, :],
                                             start=(jb == 0),
                                             stop=(jb == njb - 1))
                        nc.vector.tensor_copy(yT_all[:, tt, h, :],
                                              s_ps[:, 0, :])

                    # out-projection for this t-tile (dense PE filler)
                    ts = slice(tt * 128, (tt + 1) * 128)
                    osb = outc.tile([128, E], F32, tag="osb")
                    for ec in range(4):
                        ops = psC.tile([128, 512], F32, tag="ops")
                        for h in range(HQ):
                            nc.tensor.matmul(
                                ops, yT_all[:, tt, h, :],
                                wo_sb[:, h, ec * 512:(ec + 1) * 512],
                                start=(h == 0), stop=(h == HQ - 1))
                        nc.vector.tensor_copy(osb[:, ec * 512:(ec + 1) * 512],
                                              ops)
                    nc.sync.dma_start(out[ts, :], osb)


def build_nc(stage=99):
    nc = bacc.Bacc("TRN2", target_bir_lowering=False, debug=False,
                   num_devices=8)
    io = {
        "xT": nc.dram_tensor("xT", [E, T], BF, kind="ExternalInput").ap(),
        "wq": nc.dram_tensor("wq", [E, HQ * D], BF, kind="ExternalInput").ap(),
        "wkvg": nc.dram_tensor("wkvg", [E, 257], BF, kind="ExternalInput").ap(),
        "wo": nc.dram_tensor("wo", [HQ * D, E], BF, kind="ExternalInput").ap(),
        "ve2": nc.dram_tensor("ve2", [T, D], BF, kind="ExternalInput").ap(),
        "cos": nc.dram_tensor("cos", [T, 128], F32, kind="ExternalInput").ap(),
        "sin": nc.dram_tensor("sin", [T, 128], F32, kind="ExternalInput").ap(),
        "masks": nc.dram_tensor("masks", [2, 128, 128], BF,
                                kind="ExternalInput").ap(),
        "ident": nc.dram_tensor("ident", [128, 128], BF,
                                kind="ExternalInput").ap(),
        "out": nc.dram_tensor("out", [T, E], F32, kind="ExternalOutput").ap(),
    }
    with tile.TileContext(nc) as tc:
        _body(tc, io)
    nc.compile()
    return nc


_NC = None


def _get_nc():
    global _NC
    if _NC is None:
        _NC = build_nc()
    return _NC


def _prep_in_maps(x, ve, cos, sin, wq, wk, wv, wo, wgate):
    x = np.asarray(x, dtype=np.float32)
    ve = np.asarray(ve, dtype=np.float32)
    cos1 = np.asarray(cos, np.float32).reshape(T, 64)
    sin1 = np.asarray(sin, np.float32).reshape(T, 64)
    cos2 = np.ascontiguousarray(np.concatenate([cos1, cos1], axis=1))
    sin2 = np.ascontiguousarray(np.concatenate([sin1, -sin1], axis=1))
    ii = np.arange(128)
    masks = np.zeros((2, 128, 128), np.float32)
    masks[0][ii[:, None] >= ii[None, :]] = -30000.0  # window edge: kill j <= i
    masks[1][ii[:, None] < ii[None, :]] = -30000.0   # causal: kill j > i
    masks = np.ascontiguousarray(masks).astype(ml_dtypes.bfloat16)
    ident = np.eye(128, dtype=ml_dtypes.bfloat16)

    xT_b = [np.ascontiguousarray(x[b].T).astype(ml_dtypes.bfloat16)
            for b in range(B)]
    in_maps = []
    for c in range(8):
        b, g = divmod(c, NKV)
        wq_c = np.ascontiguousarray(
            wq[g * 512:(g + 1) * 512, :].T).astype(ml_dtypes.bfloat16)
        wk_c = wk[g * 128:(g + 1) * 128, :].T
        wv_c = wv[g * 128:(g + 1) * 128, :].T
        gcol = np.zeros((E, 1), np.float32)
        gcol[:32, 0] = wgate[g]
        wkvg_c = np.ascontiguousarray(
            np.concatenate([wk_c, wv_c, gcol], axis=1)).astype(
                ml_dtypes.bfloat16)
        wo_c = np.ascontiguousarray(
            wo[:, g * 512:(g + 1) * 512].T).astype(ml_dtypes.bfloat16)
        ve2_c = np.ascontiguousarray(
            2.0 * ve[b, :, g * 128:(g + 1) * 128]).astype(ml_dtypes.bfloat16)
        in_maps.append({
            "xT": xT_b[b], "wq": wq_c, "wkvg": wkvg_c, "wo": wo_c,
            "ve2": ve2_c, "cos": cos2, "sin": sin2, "masks": masks,
            "ident": ident,
        })
    return in_maps


def kernel(x, ve, cos, sin, wq, wk, wv, wo, wgate, window_size=512,
           _trace=False):
    assert int(window_size) == W, f"kernel hardcodes window {W}"
    wq = np.asarray(wq, np.float32)
    wk = np.asarray(wk, np.float32)
    wv = np.asarray(wv, np.float32)
    wo = np.asarray(wo, np.float32)
    wgate = np.asarray(wgate, np.float32)
    in_maps = _prep_in_maps(x, ve, cos, sin, wq, wk, wv, wo, wgate)
    nc = _get_nc()
    res = bass_utils.run_bass_kernel_spmd(
        nc, in_maps, core_ids=list(range(8)), trace=_trace)
    out = np.empty((B, T, E), np.float32)
    for b in range(B):
        acc = res.results[b * NKV]["out"].astype(np.float32).copy()
        for g in range(1, NKV):
            acc += res.results[b * NKV + g]["out"]
        out[b] = acc
    if _trace:
        kernel.last_results = res
    return out


# revision 10
# speedup vs baseline: 1.1726x; 1.1726x over previous
"""Trainium2 Bass kernel for NanochatAttention (sliding-window GQA attention).

Sharding: 8 cores = (batch b in {0,1}) x (kv-group g in {0..3}).
Each core handles one batch's full sequence for one KV head and its 4 Q heads:
projections, RoPE + QK RMS-norm, value-embedding gate, 512-window causal
attention, and the row-parallel out-projection slice -> partial [T, E] output.
Host sums the 4 partials per batch at unshard time.

Phases (keeps the PE clock warm and ACT tables stable):
  A: projections (PSUM) -> RoPE/RMS stats -> q/k transposes to [d, t].
     Input DMAs are split per contraction slab so the first projection chain
     rides the DMA wave instead of waiting for the full 11 MB.
  B+C merged: attention per (t-tile, head) -- scores + additive-mask matmuls
     -> exp (scale = per-row qscale, row sums via accum) -> P transposed AND
     divided by rowsum in one matmul against diag(1/s) -> PV -> yT [d, t];
     then immediately the out-projection for the finished t-tile (dense PE
     filler work that hides the exp/copy latency chain) -> DMA out.
"""

import numpy as np
import ml_dtypes

import concourse.bass as bass
import concourse.bacc as bacc
import concourse.tile as tile
from concourse import mybir
from concourse import bass_utils

BF = mybir.dt.bfloat16
F32 = mybir.dt.float32
AF = mybir.ActivationFunctionType
ALU = mybir.AluOpType

B = 2
T = 2048
E = 2048
D = 128          # head dim
HQ = 4           # q heads per core (one kv group)
NKV = 4
NT = T // 128    # 16 t-tiles
NE = E // 128    # 16 e-tiles
W = 512          # sliding window
NJB = W // 128   # history blocks
EPS = float(np.finfo(np.float32).eps)
SQRT_D = float(np.sqrt(128.0))


def _bcast_mid(ap, n):
    """Insert a step-0 dim after the partition dim: [p, w] -> [p, n, w]."""
    return bass.AP(tensor=ap.tensor, offset=ap.offset,
                   ap=[ap.ap[0], [0, n], *ap.ap[1:]])


def _half_swap(ap2d, nmid):
    """[p, nmid*128] -> [p, nmid, 2, 64] view with the 64-halves swapped."""
    return bass.AP(tensor=ap2d.tensor, offset=ap2d.offset + 64,
                   ap=[ap2d.ap[0], [128, nmid], [-64, 2], [1, 64]])


def _body(tc, io):
    nc = tc.nc
    xT, wq, wkvg, wo, ve2, cosd, sind, masks, ident, out = (
        io["xT"], io["wq"], io["wkvg"], io["wo"], io["ve2"], io["cos"],
        io["sin"], io["masks"], io["ident"], io["out"])

    with (
        tc.tile_pool(name="const", bufs=1) as cpool,
        tc.tile_pool(name="state", bufs=1) as state,
    ):
        # ---- small constants first (unblock tile-0 work) ----------------
        ident_sb = cpool.tile([128, 128], BF, tag="ident")
        nc.sync.dma_start(ident_sb, ident)
        masks_sb = cpool.tile([128, 2, 128], BF, tag="masks")
        nc.sync.dma_start(masks_sb, masks.rearrange("m p j -> p m j"))
        # cos_ext = [cos, cos], sin_ext = [sin, -sin]  (for 3-op RoPE)
        cos_sb = cpool.tile([128, NT, 128], F32, tag="cos")
        nc.sync.dma_start(cos_sb, cosd.rearrange("(t p) h -> p t h", p=128))
        sin_sb = cpool.tile([128, NT, 128], F32, tag="sin")
        nc.sync.dma_start(sin_sb, sind.rearrange("(t p) h -> p t h", p=128))

        # ---- weight + activation loads ----------------------------------
        wq_sb = cpool.tile([128, NE, HQ * D], BF, tag="wq")
        nc.sync.dma_start(wq_sb, wq.rearrange("(e p) f -> p e f", p=128))
        wkvg_sb = cpool.tile([128, NE, 257], BF, tag="wkvg")
        nc.sync.dma_start(wkvg_sb, wkvg.rearrange("(e p) f -> p e f", p=128))
        with tc.tile_pool(name="xp", bufs=1) as xp:
            xT_sb = xp.tile([128, NE, T], BF, tag="xT")
            for e in range(NE):
                nc.sync.dma_start(xT_sb[:, e, :], xT[e * 128:(e + 1) * 128, :])

            ve_sb = cpool.tile([128, NT, D], BF, tag="ve")
            nc.sync.dma_start(ve_sb, ve2.rearrange("(t p) d -> p t d", p=128))
            wo_sb = cpool.tile([128, HQ, E], BF, tag="wo")
            nc.sync.dma_start(wo_sb, wo.rearrange("(h p) e -> p h e", p=128))

            # ---- whole-sequence state ------------------------------------
            qkT_all = state.tile([128, NT, HQ, 128], BF, tag="qkT")
            kT_all = state.tile([128, NT, 128], BF, tag="kT")
            v_all = state.tile([128, NT, D], BF, tag="v")
            yT_all = state.tile([128, NT, HQ, 128], BF, tag="yT")
            sc_all = state.tile([128, NT, HQ + 1], F32, tag="sc")
            vstg = state.tile([128, NT, 129], BF, tag="vstg")

            # ============ PHASE A: proj + rope + rms + transposes =========
            with (
                tc.tile_pool(name="workA", bufs=2) as work,
                tc.tile_pool(name="psA_q", bufs=2, space="PSUM") as psA_q,
                tc.tile_pool(name="psA_kvg", bufs=2, space="PSUM") as psA_kvg,
                tc.tile_pool(name="psA_tr", bufs=2, space="PSUM") as psA_tr,
            ):
                for tt in range(NT):
                    ts = slice(tt * 128, (tt + 1) * 128)
                    psq = psA_q.tile([128, HQ * D], F32, tag="psq")
                    pskvg = psA_kvg.tile([128, 257], F32, tag="pskvg")
                    for e in range(NE):
                        nc.tensor.matmul(psq, xT_sb[:, e, ts], wq_sb[:, e, :],
                                         start=(e == 0), stop=(e == NE - 1))
                    for e in range(NE):
                        nc.tensor.matmul(pskvg, xT_sb[:, e, ts],
                                         wkvg_sb[:, e, :],
                                         start=(e == 0), stop=(e == NE - 1))

                    # RoPE: ro = q*cosE + halfswap(q)*sinE   (3 ops each)
                    cosq = _bcast_mid(cos_sb[:, tt, :], HQ)
                    sinq = _bcast_mid(sin_sb[:, tt, :], HQ)
                    qro = work.tile([128, HQ, D], F32, tag="qro")
                    rb = work.tile([128, HQ, D], F32, tag="rb")
                    nc.vector.tensor_mul(
                        qro, psq.rearrange("p (h d) -> p h d", h=HQ), cosq)
                    nc.vector.tensor_mul(
                        rb.rearrange("p h (s x) -> p h s x", s=2),
                        _half_swap(psq[:], HQ), sinq.rearrange(
                            "p h (s x) -> p h s x", s=2))
                    nc.vector.tensor_add(qro, qro, rb)

                    kro = work.tile([128, D], F32, tag="kro")
                    kb = work.tile([128, D], F32, tag="kb")
                    nc.vector.tensor_mul(kro, pskvg[:, 0:128],
                                         cos_sb[:, tt, :])
                    nc.vector.tensor_mul(
                        kb.rearrange("p (s x) -> p s x", s=2),
                        _half_swap(pskvg[:, 0:128], 1),
                        sin_sb[:, tt, :].rearrange("p (s x) -> p s x", s=2))
                    nc.vector.tensor_add(kro, kro, kb)

                    # RMS sums of squares (ACT, one table) then rsqrt
                    sq = work.tile([128, (HQ + 1) * D], F32, tag="sq")
                    qkss = work.tile([128, HQ + 1], F32, tag="qkss")
                    for h in range(HQ):
                        nc.scalar.activation(sq[:, h * D:(h + 1) * D],
                                             qro[:, h, :], AF.Square,
                                             accum_out=qkss[:, h:h + 1])
                    nc.scalar.activation(sq[:, HQ * D:], kro, AF.Square,
                                         accum_out=qkss[:, HQ:HQ + 1])
                    tmp5 = work.tile([128, HQ + 1], F32, tag="tmp5")
                    nc.vector.tensor_scalar_add(tmp5, qkss,
                                                float(128.0 * EPS))
                    rec5 = work.tile([128, HQ + 1], F32, tag="rec5")
                    nc.vector.reciprocal(rec5, tmp5)
                    nc.scalar.activation(sc_all[:, tt, :], rec5, AF.Sqrt)

                    # krms = kro * sc_k * sqrt(128)  (bf16)
                    krms = work.tile([128, D], BF, tag="krms")
                    nc.vector.tensor_scalar(krms, kro,
                                            sc_all[:, tt, HQ:HQ + 1],
                                            SQRT_D, op0=ALU.mult,
                                            op1=ALU.mult)
                    # stage v-proj + gate logit (cols 128:257, contiguous)
                    nc.vector.tensor_copy(vstg[:, tt, :], pskvg[:, 128:257])

                    # cast q to bf16 (gpsimd), transpose q/k to [d, t] on PE
                    qbf = work.tile([128, HQ * D], BF, tag="qbf")
                    nc.gpsimd.tensor_copy(qbf,
                                          qro.rearrange("p h d -> p (h d)"))
                    trp = psA_tr.tile([128, HQ + 1, 128], F32, tag="trp")
                    for h in range(HQ):
                        nc.tensor.matmul(trp[:, h, :],
                                         qbf[:, h * D:(h + 1) * D],
                                         ident_sb, start=True, stop=True)
                    nc.tensor.matmul(trp[:, HQ, :], krms, ident_sb,
                                     start=True, stop=True)
                    nc.vector.tensor_copy(
                        qkT_all[:, tt, :, :].rearrange("p a b -> p (a b)"),
                        trp[:, 0:HQ, :].rearrange("p a b -> p (a b)"))
                    nc.vector.tensor_copy(kT_all[:, tt, :], trp[:, HQ, :])

        # ---- A->B boundary: batched sigmoid + v assembly ----------------
        with tc.tile_pool(name="bnd", bufs=1) as bnd:
            sig_all = bnd.tile([128, NT], F32, tag="sig")
            nc.scalar.activation(sig_all, vstg[:, :, 128], AF.Sigmoid)
            for tt in range(NT):
                nc.vector.scalar_tensor_tensor(
                    v_all[:, tt, :], ve_sb[:, tt, :], sig_all[:, tt:tt + 1],
                    vstg[:, tt, 0:128], op0=ALU.mult, op1=ALU.add)

            # ========= PHASE B+C: attention + out-projection =============
            with (
                tc.tile_pool(name="attn", bufs=4) as attn,
                tc.tile_pool(name="outc", bufs=2) as outc,
                tc.tile_pool(name="psB", bufs=4, space="PSUM") as psB,
            ):
                for tt in range(NT):
                    njb = min(tt, NJB) + 1
                    jb0 = tt - (njb - 1)
                    nhist = njb - 1
                    for h in range(HQ):
                        s_ps = psB.tile([128, NJB + 1, 128], F32, tag="spt")
                        qT_h = qkT_all[:, tt, h, :]
                        if tt >= NJB:
                            nc.tensor.matmul(s_ps[:, 0, :], qT_h,
                                             kT_all[:, jb0, :],
                                             start=True, stop=False)
                            nc.tensor.matmul(s_ps[:, 0, :], ident_sb,
                                             masks_sb[:, 0, :],
                                             start=False, stop=True)
                            if nhist > 1:
                                nc.tensor.matmul(
                                    s_ps[:, 1:nhist, :].rearrange(
                                        "p a b -> p (a b)"),
                                    qT_h,
                                    kT_all[:, jb0 + 1:tt, :].rearrange(
                                        "p a b -> p (a b)"),
                                    start=True, stop=True)
                        elif nhist > 0:
                            nc.tensor.matmul(
                                s_ps[:, 0:nhist, :].rearrange(
                                    "p a b -> p (a b)"),
                                qT_h,
                                kT_all[:, jb0:tt, :].rearrange(
                                    "p a b -> p (a b)"),
                                start=True, stop=True)
                        nc.tensor.matmul(s_ps[:, nhist, :], qT_h,
                                         kT_all[:, tt, :],
                                         start=True, stop=False)
                        nc.tensor.matmul(s_ps[:, nhist, :], ident_sb,
                                         masks_sb[:, 1, :],
                                         start=False, stop=True)

                        pexp = attn.tile([128, NJB + 1, 128], BF, tag="pexp")
                        ssum = attn.tile([128, 1], F32, tag="ssum")
                        nc.scalar.activation(
                            pexp[:, 0:njb, :].rearrange("p a b -> p (a b)"),
                            s_ps[:, 0:njb, :].rearrange("p a b -> p (a b)"),
                            AF.Exp, scale=sc_all[:, tt, h:h + 1],
                            accum_out=ssum)
                        rsum = attn.tile([128, 1], F32, tag="rsum")
                        nc.vector.reciprocal(rsum, ssum)
                        diag = attn.tile([128, 128], BF, tag="diag")
                        nc.vector.tensor_scalar_mul(diag, ident_sb, rsum)

                        # PT[j, i] = P[i, j]/s(i): overwrite S slot per block
                        for jb in range(njb):
                            nc.tensor.matmul(s_ps[:, jb, :], pexp[:, jb, :],
                                             diag, start=True, stop=True)
                        pt_sb = attn.tile([128, NJB + 1, 128], BF, tag="ptsb")
                        nc.vector.tensor_copy(
                            pt_sb[:, 0:njb, :].rearrange("p a b -> p (a b)"),
                            s_ps[:, 0:njb, :].rearrange("p a b -> p (a b)"))
                        # PV: yT[d, i] accumulated over j-blocks in slot bank0
                        for jb in range(njb):
                            nc.tensor.matmul(s_ps[:, 0, :],
                                             v_all[:, jb0 + jb, :],
                                             pt_sb[:, jb, :],
                                             start=(jb == 0),
                                             stop=(jb == njb - 1))
                        nc.vector.tensor_copy(yT_all[:, tt, h, :],
                                              s_ps[:, 0, :])

                    # out-projection for this t-tile (dense PE filler)
                    ts = slice(tt * 128, (tt + 1) * 128)
                    osb = outc.tile([128, E], F32, tag="osb")
                    for ec in range(4):
                        ops_t = psB.tile([128, NJB + 1, 128], F32,
                                         tag="spt")
                        ops = ops_t[:, 0:4, :].rearrange("p a b -> p (a b)")
                        for h in range(HQ):
                            nc.tensor.matmul(
                                ops, yT_all[:, tt, h, :],
                                wo_sb[:, h, ec * 512:(ec + 1) * 512],
                                start=(h == 0), stop=(h == HQ - 1))
                        nc.vector.tensor_copy(osb[:, ec * 512:(ec + 1) * 512],
                                              ops)
                    nc.sync.dma_start(out[ts, :], osb)


def build_nc(stage=99):
    nc = bacc.Bacc("TRN2", target_bir_lowering=False, debug=False,
                   num_devices=8)
    io = {
        "xT": nc.dram_tensor("xT", [E, T], BF, kind="ExternalInput").ap(),
        "wq": nc.dram_tensor("wq", [E, HQ * D], BF, kind="ExternalInput").ap(),
        "wkvg": nc.dram_tensor("wkvg", [E, 257], BF, kind="ExternalInput").ap(),
        "wo": nc.dram_tensor("wo", [HQ * D, E], BF, kind="ExternalInput").ap(),
        "ve2": nc.dram_tensor("ve2", [T, D], BF, kind="ExternalInput").ap(),
        "cos": nc.dram_tensor("cos", [T, 128], F32, kind="ExternalInput").ap(),
        "sin": nc.dram_tensor("sin", [T, 128], F32, kind="ExternalInput").ap(),
        "masks": nc.dram_tensor("masks", [2, 128, 128], BF,
                                kind="ExternalInput").ap(),
        "ident": nc.dram_tensor("ident", [128, 128], BF,
                                kind="ExternalInput").ap(),
        "out": nc.dram_tensor("out", [T, E], F32, kind="ExternalOutput").ap(),
    }
    with tile.TileContext(nc) as tc:
        _body(tc, io)
    nc.compile()
    return nc


_NC = None


def _get_nc():
    global _NC
    if _NC is None:
        _NC = build_nc()
    return _NC


def _prep_in_maps(x, ve, cos, sin, wq, wk, wv, wo, wgate):
    x = np.asarray(x, dtype=np.float32)
    ve = np.asarray(ve, dtype=np.float32)
    cos1 = np.asarray(cos, np.float32).reshape(T, 64)
    sin1 = np.asarray(sin, np.float32).reshape(T, 64)
    cos2 = np.ascontiguousarray(np.concatenate([cos1, cos1], axis=1))
    sin2 = np.ascontiguousarray(np.concatenate([sin1, -sin1], axis=1))
    ii = np.arange(128)
    masks = np.zeros((2, 128, 128), np.float32)
    masks[0][ii[:, None] >= ii[None, :]] = -30000.0  # window edge: kill j <= i
    masks[1][ii[:, None] < ii[None, :]] = -30000.0   # causal: kill j > i
    masks = np.ascontiguousarray(masks).astype(ml_dtypes.bfloat16)
    ident = np.eye(128, dtype=ml_dtypes.bfloat16)

    xT_b = [np.ascontiguousarray(x[b].T).astype(ml_dtypes.bfloat16)
            for b in range(B)]
    in_maps = []
    for c in range(8):
        b, g = divmod(c, NKV)
        wq_c = np.ascontiguousarray(
            wq[g * 512:(g + 1) * 512, :].T).astype(ml_dtypes.bfloat16)
        wk_c = wk[g * 128:(g + 1) * 128, :].T
        wv_c = wv[g * 128:(g + 1) * 128, :].T
        gcol = np.zeros((E, 1), np.float32)
        gcol[:32, 0] = wgate[g]
        wkvg_c = np.ascontiguousarray(
            np.concatenate([wk_c, wv_c, gcol], axis=1)).astype(
                ml_dtypes.bfloat16)
        wo_c = np.ascontiguousarray(
            wo[:, g * 512:(g + 1) * 512].T).astype(ml_dtypes.bfloat16)
        ve2_c = np.ascontiguousarray(
            2.0 * ve[b, :, g * 128:(g + 1) * 128]).astype(ml_dtypes.bfloat16)
        in_maps.append({
            "xT": xT_b[b], "wq": wq_c, "wkvg": wkvg_c, "wo": wo_c,
            "ve2": ve2_c, "cos": cos2, "sin": sin2, "masks": masks,
            "ident": ident,
        })
    return in_maps


def kernel(x, ve, cos, sin, wq, wk, wv, wo, wgate, window_size=512,
           _trace=False):
    assert int(window_size) == W, f"kernel hardcodes window {W}"
    wq = np.asarray(wq, np.float32)
    wk = np.asarray(wk, np.float32)
    wv = np.asarray(wv, np.float32)
    wo = np.asarray(wo, np.float32)
    wgate = np.asarray(wgate, np.float32)
    in_maps = _prep_in_maps(x, ve, cos, sin, wq, wk, wv, wo, wgate)
    nc = _get_nc()
    res = bass_utils.run_bass_kernel_spmd(
        nc, in_maps, core_ids=list(range(8)), trace=_trace)
    out = np.empty((B, T, E), np.float32)
    for b in range(B):
        acc = res.results[b * NKV]["out"].astype(np.float32).copy()
        for g in range(1, NKV):
            acc += res.results[b * NKV + g]["out"]
        out[b] = acc
    if _trace:
        kernel.last_results = res
    return out
